# revision 32
# baseline (speedup 1.0000x reference)
"""Trainium2 Bass kernel for nn_CorrTrajBlock (sparse_attention).

Data-parallel over batch B=8 across 8 NeuronCores; one sample per core.

Per-core pipeline (C=512, T=8, H=W=28, HW=784, S=T*HW=6272, R=64, K=4,
Cq=128, P=T*R=512):
  1. template_p = w_reduce_eff @ x[:, 0]       (f32r matmul, 64x784)
     spt_inds = argmax over HW                 (DVE max/max_index)
  2. tres via on-chip one-hot selection (iota IS_EQUAL + f32r matmul
     against frame-0 x_sc chunks) - no DMA on the argmax->tres path
  3. affinity = tres^T @ x per t               (f32r matmul, 64x784 each)
     topk4 per (r, t) over HW                  (DVE max/max_index)
  4. topk idx wrapped layout built on-chip (PE fp32 transposes + i16
     copies), replicated 8x16 partitions; 4 bf16 dma_gathers of 512 rows
     (640 bf16 each, coords baked at cols 512:514)
  5. fuse = w_proj_eff @ [traj; coords] (bf16) -> max over k -> +bias2
     tc = relu(conv_t(fuse) + bias3) (bf16); tcT = (p, c) via PE transp
     points = sum_k traj (DVE, fp32) -> pts (c, p) f32r via PE transp
  6. per s-tile: z[s, p] = x-slice^T @ pts     (f32r, N=512)
     e = exp((z - max)/4) bf16 + accum denominator (ACT, bias/accum_out)
     eT via 4 PE transposes; prop[s, c] = eT^T @ tcT (bf16)
     out[s, c] = prop * (1/d) + x_sc           (one DVE stt pass)
     output written (S, C); host transposes back to (C, T, H, W).
"""
import sys

sys.path.insert(0, "/opt/trn_rl_repo")

import numpy as np
import concourse.bass as bass
import concourse.mybir as mybir
import concourse.tile as tile
from concourse import bacc
from concourse.bass_utils import run_bass_kernel_spmd

F32 = mybir.dt.float32
F32R = mybir.dt.float32r
BF16 = mybir.dt.bfloat16
I16 = mybir.dt.int16
I32 = mybir.dt.int32
U32 = mybir.dt.uint32
AF = mybir.ActivationFunctionType
ALU = mybir.AluOpType
AX = mybir.AxisListType

B, C, T, H, W = 8, 512, 8, 28, 28
HW = H * W            # 784
S = T * HW            # 6272
R = 64
K = 4
Cq = 128
P = T * R             # 512
CC = C // 128         # 4
CE = 576              # (unused fp32 row pad, kept for reference)
CEB = 640             # bf16 gather row: 512 x + 2 coords + pad (256B align)
NST = S // 128        # 49 s-tiles
NCH = 13              # s-chunks: 12 x 512 + 1 x 128

# affinity matmul dtype: f32r (fast) vs f32 (exact baseline fallback)
AFF_F32R = True
# pass 16-partition idx tiles to dma_gather directly (no 8x replication)
NO_REPL = False

_CACHED = {}


def build_nc():
    nc = bacc.Bacc("TRN2", debug=False)

    X_CS = nc.dram_tensor("x_cs", [C, S], F32, kind="ExternalInput").ap()
    X_SC = nc.dram_tensor("x_sc", [S, C], F32, kind="ExternalInput").ap()
    X_SB = nc.dram_tensor("x_sb", [S, CEB], BF16, kind="ExternalInput").ap()
    WRT = nc.dram_tensor("wrT", [C, R], F32, kind="ExternalInput").ap()
    WPT = nc.dram_tensor("wpT", [C, Cq], BF16, kind="ExternalInput").ap()
    WPC = nc.dram_tensor("wpc", [2, Cq], BF16, kind="ExternalInput").ap()
    WTT = nc.dram_tensor("wtT", [3, Cq, C], BF16, kind="ExternalInput").ap()
    B2 = nc.dram_tensor("b2", [Cq, 1], F32, kind="ExternalInput").ap()
    B3 = nc.dram_tensor("b3", [CC, 128], F32, kind="ExternalInput").ap()
    IDB = nc.dram_tensor("identbf", [128, 128], BF16, kind="ExternalInput").ap()
    IDF = nc.dram_tensor("identf", [128, 128], F32, kind="ExternalInput").ap()
    IOTA = nc.dram_tensor("iota128", [128, 1], F32, kind="ExternalInput").ap()
    OUT = nc.dram_tensor("out_sc", [S, C], F32, kind="ExternalOutput").ap()


    Xr = X_CS.rearrange("(cc p) s -> p cc s", p=128)
    XSCr = X_SC.rearrange("(n p) c -> p n c", p=128)
    OUTr = OUT.rearrange("(n p) c -> p n c", p=128)

    with tile.TileContext(nc) as tc:
        import contextlib
        ctx = contextlib.ExitStack()
        pers = ctx.enter_context(tc.tile_pool(name="pers", bufs=1))
        sb = ctx.enter_context(tc.tile_pool(name="sb", bufs=2))
        sb3 = ctx.enter_context(tc.tile_pool(name="sb3", bufs=3))
        ps = ctx.enter_context(tc.tile_pool(name="ps", bufs=3, space="PSUM"))
        pstp = ctx.enter_context(tc.tile_pool(name="pstp", bufs=2, space="PSUM"))
        pspr = ctx.enter_context(tc.tile_pool(name="pspr", bufs=3, space="PSUM"))

        # ---- persistent loads: frame 0 + weights first ----
        wrT_t = pers.tile([128, CC, R], F32R, tag="wrT")
        nc.sync.dma_start(out=wrT_t,
                          in_=WRT.rearrange("(cc p) r -> p cc r", p=128).bitcast(F32R))
        xc = pers.tile([128, CC, S], F32R, tag="xc")
        Xrr = Xr.bitcast(F32R)
        nc.sync.dma_start(out=xc[:, :, 0:392], in_=Xrr[:, :, 0:392])
        nc.sync.dma_start(out=xc[:, :, 392:HW], in_=Xrr[:, :, 392:HW])
        xcr = xc
        wpT_t = pers.tile([128, CC, Cq], BF16, tag="wpT")
        nc.sync.dma_start(out=wpT_t, in_=WPT.rearrange("(cc p) q -> p cc q", p=128))
        wpc_t = pers.tile([2, Cq], BF16, tag="wpc")
        nc.sync.dma_start(out=wpc_t, in_=WPC)
        wtT_t = pers.tile([128, 3, C], BF16, tag="wtT")
        nc.sync.dma_start(out=wtT_t, in_=WTT.rearrange("d p c -> p d c"))
        b2_t = pers.tile([128, 1], F32, tag="b2")
        nc.sync.dma_start(out=b2_t, in_=B2)
        b3_t = pers.tile([128, CC], F32, tag="b3")
        nc.sync.dma_start(out=b3_t, in_=B3.rearrange("cc p -> p cc"))
        idb_t = pers.tile([128, 128], BF16, tag="idb")
        nc.sync.dma_start(out=idb_t, in_=IDB)
        idf_t = pers.tile([128, 128], F32, tag="idf")
        nc.sync.dma_start(out=idf_t, in_=IDF)
        iota_t = pers.tile([128, 1], F32, tag="iota")
        nc.sync.dma_start(out=iota_t, in_=IOTA)
        xf0T = pers.tile([112, 7, C], F32R, tag="xf0T")
        nc.sync.dma_start(
            out=xf0T,
            in_=X_SC[0:HW, 0:C].rearrange("(j p) c -> p j c", p=112).bitcast(F32R))
        onesw = pers.tile([1, 128], F32, tag="onesw")
        nc.vector.memset(onesw, 1.0)

        # ---- phase 1: template + argmax ----
        tpl_sb = sb3.tile([64, HW], F32, tag="aff")
        for h in range(2):
            tp_ps = ps.tile([64, 392], F32, tag="mm")
            for cc in range(CC):
                nc.tensor.matmul(tp_ps,
                                 lhsT=wrT_t[:, cc, :],
                                 rhs=xcr[:, cc, h * 392:(h + 1) * 392],
                                 start=(cc == 0), stop=(cc == CC - 1))
            nc.scalar.activation(tpl_sb[:, h * 392:(h + 1) * 392], tp_ps, AF.Copy)
        for wi in range(16):
            wps = ps.tile([64, 392], F32, tag="mm")
            nc.tensor.matmul(wps, lhsT=wrT_t[:, wi % CC, :],
                             rhs=xcr[:, wi % CC, 0:392], start=True, stop=True)
        tmx = pers.tile([64, 8], F32, tag="tmx")
        tmi = pers.tile([64, 8], U32, tag="tmi")
        nc.vector.max(out=tmx, in_=tpl_sb)
        nc.vector.max_index(out=tmi, in_max=tmx, in_values=tpl_sb)
        spt_f = pers.tile([64, 1], F32, tag="sptf")
        nc.vector.tensor_copy(spt_f, tmi[:, 0:1])

        # tres via on-chip one-hot selection: no DMA in the argmax->tres path.
        # spt row -> broadcast to 112 partitions (ones-matmul), compare with
        # iota per 112-row chunk of frame-0 x_sc, one-hot matmul selects cols.
        tp_s = pstp.tile([1, 512], F32, tag="tp")
        nc.tensor.transpose(tp_s[:, 0:64], spt_f, idf_t[0:64, 0:64])
        t1s = pers.tile([1, 64], F32, tag="t1s")
        nc.vector.tensor_copy(t1s, tp_s[:, 0:64])
        rep_ps = pstp.tile([128, 512], F32, tag="tp")
        nc.tensor.matmul(rep_ps[:, 0:64], lhsT=onesw, rhs=t1s,
                         start=True, stop=True)
        rep = pers.tile([128, 64], F32, tag="rep")
        nc.scalar.activation(rep, rep_ps[:, 0:64], AF.Copy)
        tresT2_ps = pstp.tile([64, 512], F32, tag="tp")
        oh = []
        for j in range(7):
            oh_j = sb.tile([112, 64], F32, tag="oh")
            nc.vector.tensor_scalar(oh_j, rep[0:112, :], float(-112 * j),
                                    iota_t[0:112, :], op0=ALU.add,
                                    op1=ALU.is_equal)
            oh_r = sb.tile([112, 64], F32R, tag="ohr")
            nc.scalar.activation(oh_r, oh_j, AF.Copy)
            nc.tensor.matmul(tresT2_ps, lhsT=oh_r, rhs=xf0T[:, j, :],
                             start=(j == 0), stop=(j == 6))
        tresT2 = pers.tile([64, C], F32, tag="tresT2")
        nc.scalar.activation(tresT2, tresT2_ps, AF.Copy)
        tres = pers.tile([128, CC, R], F32R, tag="tres")
        for cc in range(CC):
            tp = pstp.tile([128, 512], F32, tag="tp")
            nc.tensor.transpose(tp[:, 0:64],
                                tresT2[:, cc * 128:(cc + 1) * 128],
                                idf_t[0:64, 0:64])
            nc.scalar.activation(tres[:, cc, :], tp[:, 0:64], AF.Copy)

        # rest of x: per-half-frame DMAs so affinity can stream per t
        for tb in range(1, T):
            for hh in range(2):
                i0 = tb * HW + hh * 392
                nc.sync.dma_start(out=xc[:, :, i0:i0 + 392],
                                  in_=Xrr[:, :, i0:i0 + 392])

        # ---- phase 2: affinity + topk (per t; 64-partition tiles) ----
        # staging layout: gstage2[r, (k t)] so that dram j = 512k + 64t + r
        gstage2 = pers.tile([64, 32], F32, tag="gstage2")
        gs2_v = gstage2.rearrange("r (k t) -> r t k", t=8)
        tres_mm = tres if AFF_F32R else tres.bitcast(F32)
        xc_mm = xcr if AFF_F32R else xc.bitcast(F32)
        for t in range(T):
            aff_sb = sb3.tile([64, HW], F32, tag="aff")
            for h in range(2):
                a_ps = ps.tile([64, 392], F32, tag="mm")
                for cc in range(CC):
                    nc.tensor.matmul(
                        a_ps,
                        lhsT=tres_mm[:, cc, :],
                        rhs=xc_mm[:, cc, t * HW + h * 392: t * HW + (h + 1) * 392],
                        start=(cc == 0), stop=(cc == CC - 1))
                nc.scalar.activation(aff_sb[:, h * 392:(h + 1) * 392], a_ps, AF.Copy)
            amx = sb3.tile([64, 8], F32, tag="amx")
            ami = sb3.tile([64, 8], U32, tag="ami")
            nc.vector.max(out=amx, in_=aff_sb)
            nc.vector.max_index(out=ami, in_max=amx, in_values=aff_sb)
            nc.vector.tensor_scalar(gs2_v[:, t, :], ami[:, 0:K],
                                    float(t * HW), None, op0=ALU.add)

        # build wrapped idx layout on-chip: gstage2[r=16rh+p16, q=8k+t]
        # -> w16[p16, 4q+rh] via PE int16 transposes, then replicate 8x.
        t1_ps = pstp.tile([32, 64], F32, tag="tp")
        nc.tensor.transpose(t1_ps, gstage2, idf_t[0:64, 0:64])
        t1 = pers.tile([32, 64], F32, tag="t1")
        nc.vector.tensor_copy(t1, t1_ps)
        w16 = pers.tile([16, 128], I16, tag="w16")
        w16v = w16.rearrange("p (q rh) -> p q rh", rh=4)
        for rh in range(4):
            wr_ps = pstp.tile([16, 32], F32, tag="tp")
            nc.tensor.transpose(wr_ps, t1[:, 16 * rh:16 * (rh + 1)],
                                idf_t[0:32, 0:32])
            nc.vector.tensor_copy(w16v[:, :, rh], wr_ps)
        if NO_REPL:
            gidx2 = w16
        else:
            gidx2 = pers.tile([128, 128], I16, tag="gidx2")
            for g in range(8):
                nc.scalar.dma_start(out=gidx2[16 * g:16 * (g + 1), :], in_=w16)

        # ---- phase 3: traj gathers, fuse, points, conv ----
        fm_f32 = pers.tile([128, P], F32, tag="fmf")
        spts = pers.tile([128, 4, P], F32, tag="spts")
        gk = []
        for k in range(K):
            gk_t = sb.tile([128, 4, CEB], BF16, tag="gk")
            gk.append(gk_t)
            if k == 0:
                for hg in range(2):
                    nc.gpsimd.dma_gather(
                        out_ap=gk_t[:, 2 * hg:2 * hg + 2, :], in_ap=X_SB,
                        idxs_ap=gidx2[:, 16 * hg:16 * hg + 16],
                        num_idxs=256, num_idxs_reg=256, elem_size=CEB)
            else:
                nc.gpsimd.dma_gather(out_ap=gk_t, in_ap=X_SB,
                                     idxs_ap=gidx2[:, k * 32:(k + 1) * 32],
                                     num_idxs=512, num_idxs_reg=512,
                                     elem_size=CEB)
            trajk = sb.tile([128, CC, P], BF16, tag="trajk")
            for cc in range(CC):
                tp = pstp.tile([128, 512], F32, tag="tp")
                tpb = tp.bitcast(BF16)
                for jb in range(4):
                    nc.tensor.transpose(tpb[:, jb * 128:(jb + 1) * 128],
                                        gk_t[:, jb, cc * 128:(cc + 1) * 128], idb_t)
                nc.scalar.activation(trajk[:, cc, :], tpb[:, 0:P], AF.Copy)
            # coords rows (gathered cols 512:514) -> (2, P)
            cd = pstp.tile([2, 512], F32, tag="tp")
            cdb = cd.bitcast(BF16)
            for jb in range(4):
                nc.tensor.transpose(cdb[:, jb * 128:(jb + 1) * 128],
                                    gk_t[:, jb, 512:514], idb_t)
            coordk = sb.tile([2, P], BF16, tag="coordk")
            nc.vector.tensor_copy(coordk, cdb[:, 0:P])
            f_ps = ps.tile([128, P], F32, tag="mm")
            for cc in range(CC):
                nc.tensor.matmul(f_ps, lhsT=wpT_t[:, cc, :], rhs=trajk[:, cc, :],
                                 start=(cc == 0), stop=False)
            nc.tensor.matmul(f_ps, lhsT=wpc_t, rhs=coordk,
                             start=False, stop=True)
            if k == 0:
                nc.scalar.activation(fm_f32, f_ps, AF.Copy)
            else:
                nc.vector.tensor_tensor(out=fm_f32, in0=fm_f32, in1=f_ps, op=ALU.max)
            if k == 1:
                nc.vector.tensor_tensor(out=spts, in0=gk[0][:, :, 0:P],
                                        in1=gk[1][:, :, 0:P], op=ALU.add)
            elif k > 1:
                nc.vector.tensor_tensor(out=spts, in0=spts,
                                        in1=gk_t[:, :, 0:P], op=ALU.add)
        fm = pers.tile([128, P], BF16, tag="fm")
        nc.vector.tensor_scalar(fm, fm_f32, b2_t, None, op0=ALU.add)

        # points = sum_k traj_k (fp32); 1/4 folded into exp scale
        pts_f = pers.tile([128, CC, P], F32R, tag="ptsf")
        for cc in range(CC):
            tp = pstp.tile([128, 512], F32, tag="tp")
            for jb in range(4):
                nc.tensor.transpose(tp[:, jb * 128:(jb + 1) * 128],
                                    spts[:, jb, cc * 128:(cc + 1) * 128], idf_t)
            nc.scalar.activation(pts_f[:, cc, :], tp, AF.Copy)
        ptsr = pts_f

        # conv over t (3 taps) + bias3 + relu -> tc bf16
        tc_bf = pers.tile([128, CC, P], BF16, tag="tcbf")
        for ct in range(CC):
            c_ps = ps.tile([128, P], F32, tag="mm")
            cs = slice(ct * 128, (ct + 1) * 128)
            nc.tensor.matmul(c_ps, lhsT=wtT_t[:, 1, cs], rhs=fm,
                             start=True, stop=False)
            nc.tensor.matmul(c_ps[:, R:P], lhsT=wtT_t[:, 0, cs], rhs=fm[:, 0:P - R],
                             start=False, stop=False)
            nc.tensor.matmul(c_ps[:, 0:P - R], lhsT=wtT_t[:, 2, cs], rhs=fm[:, R:P],
                             start=False, stop=True)
            nc.scalar.activation(tc_bf[:, ct, :], c_ps, AF.Relu,
                                 bias=b3_t[:, ct:ct + 1])
        for wi in range(8):
            wps = ps.tile([64, 392], F32, tag="mm")
            nc.tensor.matmul(wps, lhsT=wrT_t[:, wi % CC, :],
                             rhs=xcr[:, wi % CC, 0:392], start=True, stop=True)
        # tcT: (p, c) layout for prop matmuls
        tcT = pers.tile([128, CC, C], BF16, tag="tcT")
        for pb in range(4):
            tp2 = pstp.tile([128, 512], BF16, tag="tp")
            for cc in range(CC):
                nc.tensor.transpose(tp2[:, cc * 128:(cc + 1) * 128],
                                    tc_bf[:, cc, pb * 128:(pb + 1) * 128], idb_t)
            nc.vector.tensor_copy(tcT[:, pb, :], tp2)

        # ---- phase 4: z (s,p) -> softmax -> prop -> out, per s-tile ----
        # z per s-tile: lhsT = x slices, rhs = pts (f32r).  exp with
        # per-tile max bias + accumulated denominator; e transposed via PE
        # (bf16) and fed back as prop lhsT; normalize + residual fused in
        # one DVE pass at the output.
        for i in [NCH - 1] + list(range(NCH - 1)):
            cw = min(P, S - i * P)
            nt = cw // 128
            for t in range(nt):
                st = 4 * i + t
                xres = sb3.tile([128, C], F32, tag="xres")
                nc.sync.dma_start(out=xres, in_=XSCr[:, st, 0:C])
                z_ps = ps.tile([128, P], F32, tag="mm")
                for cc in range(CC):
                    nc.tensor.matmul(z_ps,
                                     lhsT=xcr[:, cc, st * 128:(st + 1) * 128],
                                     rhs=ptsr[:, cc, :],
                                     start=(cc == 0), stop=(cc == CC - 1))
                nm = sb3.tile([128, 1], F32, tag="nm")
                nc.vector.tensor_reduce(nm, z_ps, axis=AX.X, op=ALU.max,
                                        negate=True)
                nm4 = sb3.tile([128, 1], F32, tag="nm4")
                nc.vector.tensor_scalar(nm4, nm, 0.25, None, op0=ALU.mult)
                e_sb = sb3.tile([128, P], BF16, tag="esb")
                dsum = sb3.tile([128, 1], F32, tag="dsum")
                nc.scalar.activation(e_sb, z_ps, AF.Exp, bias=nm4, scale=0.25,
                                     accum_out=dsum)
                eT_ps = pstp.tile([128, 512], BF16, tag="tp")
                for pb in range(4):
                    nc.tensor.transpose(eT_ps[:, pb * 128:(pb + 1) * 128],
                                        e_sb[:, pb * 128:(pb + 1) * 128], idb_t)
                eT = sb3.tile([128, 512], BF16, tag="eT")
                nc.scalar.activation(eT, eT_ps, AF.Copy)
                pr = pspr.tile([128, C], F32, tag="pr")
                for pb in range(4):
                    nc.tensor.matmul(pr, lhsT=eT[:, pb * 128:(pb + 1) * 128],
                                     rhs=tcT[:, pb, :],
                                     start=(pb == 0), stop=(pb == CC - 1))
                rd = sb3.tile([128, 1], F32, tag="rd")
                nc.vector.reciprocal(rd, dsum)
                osb = sb3.tile([128, C], F32, tag="osb")
                nc.vector.scalar_tensor_tensor(
                    osb, in0=pr, scalar=rd, in1=xres,
                    op0=ALU.mult, op1=ALU.add)
                nc.sync.dma_start(out=OUTr[:, st, :], in_=osb)
        ctx.close()
    nc.compile()
    return nc


def _host_prep(inputs):
    eps = 1e-5
    f32 = np.float32
    import ml_dtypes
    bf16 = ml_dtypes.bfloat16
    x = np.asarray(inputs["input"], f32)                       # (B,C,T,H,W)
    s1 = np.asarray(inputs["bn1_gamma"]) / np.sqrt(np.asarray(inputs["bn1_var"]) + eps)
    wrT = (np.asarray(inputs["w_reduce"], f32) * s1[:, None]).T.astype(f32)
    s2 = np.asarray(inputs["bn2_gamma"]) / np.sqrt(np.asarray(inputs["bn2_var"]) + eps)
    wp = np.asarray(inputs["w_proj"], f32) * s2[:, None]       # (Cq, C+2)
    b2 = (np.asarray(inputs["bn2_beta"])
          - np.asarray(inputs["bn2_mean"]) * s2).astype(f32)
    s3 = np.asarray(inputs["bn3_gamma"]) / np.sqrt(np.asarray(inputs["bn3_var"]) + eps)
    wt = np.asarray(inputs["w_t"], f32)[:, :, :, 0] * s3[:, None, None]  # (C,Cq,3)
    b3 = (np.asarray(inputs["bn3_beta"])
          - np.asarray(inputs["bn3_mean"]) * s3).astype(f32)
    common = {
        "wrT": np.ascontiguousarray(wrT),
        "wpT": np.ascontiguousarray(wp[:, :C].T.astype(bf16)),
        "wpc": np.ascontiguousarray(wp[:, C:].T.astype(bf16)),
        "wtT": np.ascontiguousarray(np.transpose(wt, (2, 1, 0)).astype(bf16)),
        "b2": b2.reshape(Cq, 1),
        "b3": b3.reshape(CC, 128),
        "identbf": np.eye(128, dtype=bf16),
        "identf": np.eye(128, dtype=f32),
        "iota128": np.arange(128, dtype=f32).reshape(128, 1),
    }
    x_cs = x.reshape(B, C, S)
    # augmented (S, CE) per-sample: x^T | row/H | col/W | zero pad
    hw_idx = np.arange(HW, dtype=f32)
    rowv = np.tile((hw_idx // W) / H, T)                       # (S,)
    colv = np.tile((hw_idx % W) / W, T)
    in_maps = []
    for b in range(B):
        m = dict(common)
        m["x_cs"] = np.ascontiguousarray(x_cs[b])
        m["x_sc"] = np.ascontiguousarray(x_cs[b].T)
        augb = np.zeros((S, CEB), bf16)
        augb[:, :C] = x_cs[b].T.astype(bf16)
        augb[:, C] = rowv.astype(bf16)
        augb[:, C + 1] = colv.astype(bf16)
        m["x_sb"] = augb
        in_maps.append(m)
    return in_maps


def kernel(**inputs) -> np.ndarray:
    if "nc" not in _CACHED:
        _CACHED["nc"] = build_nc()
    nc = _CACHED["nc"]
    in_maps = _host_prep(inputs)
    res = run_bass_kernel_spmd(nc, in_maps, list(range(B)))
    out = np.stack([res.results[b]["out_sc"].T for b in range(B)], axis=0)
    return out.reshape(B, C, T, H, W).astype(np.float32)


# revision 33
# speedup vs baseline: 1.0132x; 1.0132x over previous
"""Trainium2 Bass kernel for nn_CorrTrajBlock (sparse_attention).

Data-parallel over batch B=8 across 8 NeuronCores; one sample per core.

Per-core pipeline (C=512, T=8, H=W=28, HW=784, S=T*HW=6272, R=64, K=4,
Cq=128, P=T*R=512):
  1. template_p = w_reduce_eff @ x[:, 0]       (f32r matmul, 64x784)
     spt_inds = argmax over HW                 (DVE max/max_index)
  2. tres via on-chip one-hot selection (iota IS_EQUAL + f32r matmul
     against frame-0 x_sc chunks) - no DMA on the argmax->tres path
  3. affinity = tres^T @ x per t               (f32r matmul, 64x784 each)
     topk4 per (r, t) over HW                  (DVE max/max_index)
  4. topk idx wrapped layout built on-chip (PE fp32 transposes + i16
     copies), replicated 8x16 partitions; 4 bf16 dma_gathers of 512 rows
     (640 bf16 each, coords baked at cols 512:514)
  5. fuse = w_proj_eff @ [traj; coords] (bf16) -> max over k -> +bias2
     tc = relu(conv_t(fuse) + bias3) (bf16); tcT = (p, c) via PE transp
     points = sum_k traj (DVE, fp32) -> pts (c, p) f32r via PE transp
  6. per s-tile: z[s, p] = x-slice^T @ pts     (f32r, N=512)
     e = exp((z - max)/4) bf16 + accum denominator (ACT, bias/accum_out)
     eT via 4 PE transposes; prop[s, c] = eT^T @ tcT (bf16)
     out[s, c] = prop * (1/d) + x_sc           (one DVE stt pass)
     output written (S, C); host transposes back to (C, T, H, W).
"""
import sys

sys.path.insert(0, "/opt/trn_rl_repo")

import numpy as np
import concourse.bass as bass
import concourse.mybir as mybir
import concourse.tile as tile
from concourse import bacc
from concourse.bass_utils import run_bass_kernel_spmd

F32 = mybir.dt.float32
F32R = mybir.dt.float32r
BF16 = mybir.dt.bfloat16
I16 = mybir.dt.int16
I32 = mybir.dt.int32
U32 = mybir.dt.uint32
AF = mybir.ActivationFunctionType
ALU = mybir.AluOpType
AX = mybir.AxisListType

B, C, T, H, W = 8, 512, 8, 28, 28
HW = H * W            # 784
S = T * HW            # 6272
R = 64
K = 4
Cq = 128
P = T * R             # 512
CC = C // 128         # 4
CE = 576              # (unused fp32 row pad, kept for reference)
CEB = 640             # bf16 gather row: 512 x + 2 coords + pad (256B align)
NST = S // 128        # 49 s-tiles
NCH = 13              # s-chunks: 12 x 512 + 1 x 128

# affinity matmul dtype: f32r (fast) vs f32 (exact baseline fallback)
AFF_F32R = True
# pass 16-partition idx tiles to dma_gather directly (no 8x replication)
NO_REPL = False

_CACHED = {}


def build_nc():
    nc = bacc.Bacc("TRN2", debug=False)

    X_CS = nc.dram_tensor("x_cs", [C, S], F32, kind="ExternalInput").ap()
    X_SC = nc.dram_tensor("x_sc", [S, C], F32, kind="ExternalInput").ap()
    X_SB = nc.dram_tensor("x_sb", [S, CEB], BF16, kind="ExternalInput").ap()
    WRT = nc.dram_tensor("wrT", [C, R], F32, kind="ExternalInput").ap()
    WPT = nc.dram_tensor("wpT", [C, Cq], BF16, kind="ExternalInput").ap()
    WPC = nc.dram_tensor("wpc", [2, Cq], BF16, kind="ExternalInput").ap()
    WTT = nc.dram_tensor("wtT", [3, Cq, C], BF16, kind="ExternalInput").ap()
    B2 = nc.dram_tensor("b2", [Cq, 1], F32, kind="ExternalInput").ap()
    B3 = nc.dram_tensor("b3", [CC, 128], F32, kind="ExternalInput").ap()
    IDB = nc.dram_tensor("identbf", [128, 128], BF16, kind="ExternalInput").ap()
    IDF = nc.dram_tensor("identf", [128, 128], F32, kind="ExternalInput").ap()
    IOTA = nc.dram_tensor("iota128", [128, 1], F32, kind="ExternalInput").ap()
    OUT = nc.dram_tensor("out_sc", [S, C], F32, kind="ExternalOutput").ap()


    Xr = X_CS.rearrange("(cc p) s -> p cc s", p=128)
    XSCr = X_SC.rearrange("(n p) c -> p n c", p=128)
    OUTr = OUT.rearrange("(n p) c -> p n c", p=128)

    with tile.TileContext(nc) as tc:
        import contextlib
        ctx = contextlib.ExitStack()
        pers = ctx.enter_context(tc.tile_pool(name="pers", bufs=1))
        sb = ctx.enter_context(tc.tile_pool(name="sb", bufs=2))
        sb3 = ctx.enter_context(tc.tile_pool(name="sb3", bufs=3))
        ps = ctx.enter_context(tc.tile_pool(name="ps", bufs=3, space="PSUM"))
        pstp = ctx.enter_context(tc.tile_pool(name="pstp", bufs=2, space="PSUM"))
        pspr = ctx.enter_context(tc.tile_pool(name="pspr", bufs=3, space="PSUM"))

        # ---- persistent loads: frame 0 + weights first ----
        wrT_t = pers.tile([128, CC, R], F32R, tag="wrT")
        nc.sync.dma_start(out=wrT_t,
                          in_=WRT.rearrange("(cc p) r -> p cc r", p=128).bitcast(F32R))
        xc = pers.tile([128, CC, S], F32R, tag="xc")
        Xrr = Xr.bitcast(F32R)
        nc.sync.dma_start(out=xc[:, :, 0:392], in_=Xrr[:, :, 0:392])
        nc.sync.dma_start(out=xc[:, :, 392:HW], in_=Xrr[:, :, 392:HW])
        xcr = xc
        wpT_t = pers.tile([128, CC, Cq], BF16, tag="wpT")
        nc.sync.dma_start(out=wpT_t, in_=WPT.rearrange("(cc p) q -> p cc q", p=128))
        wpc_t = pers.tile([2, Cq], BF16, tag="wpc")
        nc.sync.dma_start(out=wpc_t, in_=WPC)
        wtT_t = pers.tile([128, 3, C], BF16, tag="wtT")
        nc.sync.dma_start(out=wtT_t, in_=WTT.rearrange("d p c -> p d c"))
        b2_t = pers.tile([128, 1], F32, tag="b2")
        nc.sync.dma_start(out=b2_t, in_=B2)
        b3_t = pers.tile([128, CC], F32, tag="b3")
        nc.sync.dma_start(out=b3_t, in_=B3.rearrange("cc p -> p cc"))
        idb_t = pers.tile([128, 128], BF16, tag="idb")
        nc.sync.dma_start(out=idb_t, in_=IDB)
        idf_t = pers.tile([128, 128], F32, tag="idf")
        nc.sync.dma_start(out=idf_t, in_=IDF)
        iota_t = pers.tile([128, 1], F32, tag="iota")
        nc.sync.dma_start(out=iota_t, in_=IOTA)
        xf0T = pers.tile([112, 7, C], F32R, tag="xf0T")
        nc.sync.dma_start(
            out=xf0T,
            in_=X_SC[0:HW, 0:C].rearrange("(j p) c -> p j c", p=112).bitcast(F32R))
        onesw = pers.tile([1, 128], F32, tag="onesw")
        nc.vector.memset(onesw, 1.0)

        # ---- phase 1: template + argmax ----
        tpl_sb = sb3.tile([64, HW], F32, tag="aff")
        for h in range(2):
            tp_ps = ps.tile([64, 392], F32, tag="mm")
            for cc in range(CC):
                nc.tensor.matmul(tp_ps,
                                 lhsT=wrT_t[:, cc, :],
                                 rhs=xcr[:, cc, h * 392:(h + 1) * 392],
                                 start=(cc == 0), stop=(cc == CC - 1))
            nc.scalar.activation(tpl_sb[:, h * 392:(h + 1) * 392], tp_ps, AF.Copy)
        for wi in range(16):
            wps = ps.tile([64, 392], F32, tag="mm")
            nc.tensor.matmul(wps, lhsT=wrT_t[:, wi % CC, :],
                             rhs=xcr[:, wi % CC, 0:392], start=True, stop=True)
        tmx = pers.tile([64, 8], F32, tag="tmx")
        tmi = pers.tile([64, 8], U32, tag="tmi")
        nc.vector.max(out=tmx, in_=tpl_sb)
        nc.vector.max_index(out=tmi, in_max=tmx, in_values=tpl_sb)
        spt_f = pers.tile([64, 1], F32, tag="sptf")
        nc.vector.tensor_copy(spt_f, tmi[:, 0:1])

        # tres via on-chip one-hot selection: no DMA in the argmax->tres path.
        # spt row -> broadcast to 112 partitions (ones-matmul), compare with
        # iota per 112-row chunk of frame-0 x_sc, one-hot matmul selects cols.
        tp_s = pstp.tile([1, 512], F32, tag="tp")
        nc.tensor.transpose(tp_s[:, 0:64], spt_f, idf_t[0:64, 0:64])
        t1s = pers.tile([1, 64], F32, tag="t1s")
        nc.vector.tensor_copy(t1s, tp_s[:, 0:64])
        rep_ps = pstp.tile([128, 512], F32, tag="tp")
        nc.tensor.matmul(rep_ps[:, 0:64], lhsT=onesw, rhs=t1s,
                         start=True, stop=True)
        rep = pers.tile([128, 64], F32, tag="rep")
        nc.scalar.activation(rep, rep_ps[:, 0:64], AF.Copy)
        tresT2_ps = pstp.tile([64, 512], F32, tag="tp")
        oh = []
        for j in range(7):
            oh_j = sb.tile([112, 64], F32, tag="oh")
            nc.vector.tensor_scalar(oh_j, rep[0:112, :], float(-112 * j),
                                    iota_t[0:112, :], op0=ALU.add,
                                    op1=ALU.is_equal)
            oh_r = sb.tile([112, 64], F32R, tag="ohr")
            nc.scalar.activation(oh_r, oh_j, AF.Copy)
            nc.tensor.matmul(tresT2_ps, lhsT=oh_r, rhs=xf0T[:, j, :],
                             start=(j == 0), stop=(j == 6))
        tresT2 = pers.tile([64, C], F32, tag="tresT2")
        nc.scalar.activation(tresT2, tresT2_ps, AF.Copy)
        tres = pers.tile([128, CC, R], F32R, tag="tres")
        for cc in range(CC):
            tp = pstp.tile([128, 512], F32, tag="tp")
            nc.tensor.transpose(tp[:, 0:64],
                                tresT2[:, cc * 128:(cc + 1) * 128],
                                idf_t[0:64, 0:64])
            nc.scalar.activation(tres[:, cc, :], tp[:, 0:64], AF.Copy)

        # rest of x: per-half-frame DMAs so affinity can stream per t
        for tb in range(1, T):
            for hh in range(2):
                i0 = tb * HW + hh * 392
                nc.sync.dma_start(out=xc[:, :, i0:i0 + 392],
                                  in_=Xrr[:, :, i0:i0 + 392])

        # ---- phase 2: affinity + topk (per t; 64-partition tiles) ----
        # staging layout: gstage2[r, (k t)] so that dram j = 512k + 64t + r
        gstage2 = pers.tile([64, 32], F32, tag="gstage2")
        gs2_v = gstage2.rearrange("r (k t) -> r t k", t=8)
        tres_mm = tres if AFF_F32R else tres.bitcast(F32)
        xc_mm = xcr if AFF_F32R else xc.bitcast(F32)
        for t in range(T):
            aff_sb = sb3.tile([64, HW], F32, tag="aff")
            for h in range(2):
                a_ps = ps.tile([64, 392], F32, tag="mm")
                for cc in range(CC):
                    nc.tensor.matmul(
                        a_ps,
                        lhsT=tres_mm[:, cc, :],
                        rhs=xc_mm[:, cc, t * HW + h * 392: t * HW + (h + 1) * 392],
                        start=(cc == 0), stop=(cc == CC - 1))
                nc.scalar.activation(aff_sb[:, h * 392:(h + 1) * 392], a_ps, AF.Copy)
            amx = sb3.tile([64, 8], F32, tag="amx")
            ami = sb3.tile([64, 8], U32, tag="ami")
            nc.vector.max(out=amx, in_=aff_sb)
            nc.vector.max_index(out=ami, in_max=amx, in_values=aff_sb)
            nc.vector.tensor_scalar(gs2_v[:, t, :], ami[:, 0:K],
                                    float(t * HW), None, op0=ALU.add)

        # build wrapped idx layout on-chip: gstage2[r=16rh+p16, q=8k+t]
        # -> w16[p16, 4q+rh] via PE int16 transposes, then replicate 8x.
        t1_ps = pstp.tile([32, 64], F32, tag="tp")
        nc.tensor.transpose(t1_ps, gstage2, idf_t[0:64, 0:64])
        t1 = pers.tile([32, 64], F32, tag="t1")
        nc.vector.tensor_copy(t1, t1_ps)
        w16 = pers.tile([16, 128], I16, tag="w16")
        w16v = w16.rearrange("p (q rh) -> p q rh", rh=4)
        for rh in range(4):
            wr_ps = pstp.tile([16, 32], F32, tag="tp")
            nc.tensor.transpose(wr_ps, t1[:, 16 * rh:16 * (rh + 1)],
                                idf_t[0:32, 0:32])
            nc.vector.tensor_copy(w16v[:, :, rh], wr_ps)
        if NO_REPL:
            gidx2 = w16
        else:
            gidx2 = pers.tile([128, 128], I16, tag="gidx2")
            for g in range(8):
                nc.scalar.dma_start(out=gidx2[16 * g:16 * (g + 1), :], in_=w16)

        # ---- phase 3: traj gathers, fuse, points, conv ----
        fm_f32 = pers.tile([128, P], F32, tag="fmf")
        spts = pers.tile([128, 4, P], F32, tag="spts")
        gk = []
        for k in range(K):
            gk_t = sb.tile([128, 4, CEB], BF16, tag="gk")
            gk.append(gk_t)
            if k == 0:
                for hg in range(2):
                    nc.gpsimd.dma_gather(
                        out_ap=gk_t[:, 2 * hg:2 * hg + 2, :], in_ap=X_SB,
                        idxs_ap=gidx2[:, 16 * hg:16 * hg + 16],
                        num_idxs=256, num_idxs_reg=256, elem_size=CEB)
            else:
                nc.gpsimd.dma_gather(out_ap=gk_t, in_ap=X_SB,
                                     idxs_ap=gidx2[:, k * 32:(k + 1) * 32],
                                     num_idxs=512, num_idxs_reg=512,
                                     elem_size=CEB)
            trajk = sb.tile([128, CC, P], BF16, tag="trajk")
            for cc in range(CC):
                tp = pstp.tile([128, 512], F32, tag="tp")
                tpb = tp.bitcast(BF16)
                for jb in range(4):
                    nc.tensor.transpose(tpb[:, jb * 128:(jb + 1) * 128],
                                        gk_t[:, jb, cc * 128:(cc + 1) * 128], idb_t)
                nc.scalar.activation(trajk[:, cc, :], tpb[:, 0:P], AF.Copy)
            # coords rows (gathered cols 512:514) -> (2, P)
            cd = pstp.tile([2, 512], F32, tag="tp")
            cdb = cd.bitcast(BF16)
            for jb in range(4):
                nc.tensor.transpose(cdb[:, jb * 128:(jb + 1) * 128],
                                    gk_t[:, jb, 512:514], idb_t)
            coordk = sb.tile([2, P], BF16, tag="coordk")
            nc.vector.tensor_copy(coordk, cdb[:, 0:P])
            f_ps = ps.tile([128, P], F32, tag="mm")
            for cc in range(CC):
                nc.tensor.matmul(f_ps, lhsT=wpT_t[:, cc, :], rhs=trajk[:, cc, :],
                                 start=(cc == 0), stop=False)
            nc.tensor.matmul(f_ps, lhsT=wpc_t, rhs=coordk,
                             start=False, stop=True)
            if k == 0:
                nc.scalar.activation(fm_f32, f_ps, AF.Copy)
            else:
                nc.vector.tensor_tensor(out=fm_f32, in0=fm_f32, in1=f_ps, op=ALU.max)
            if k == 1:
                nc.vector.tensor_tensor(out=spts, in0=gk[0][:, :, 0:P],
                                        in1=gk[1][:, :, 0:P], op=ALU.add)
            elif k > 1:
                nc.vector.tensor_tensor(out=spts, in0=spts,
                                        in1=gk_t[:, :, 0:P], op=ALU.add)
        fm = pers.tile([128, P], BF16, tag="fm")
        nc.vector.tensor_scalar(fm, fm_f32, b2_t, None, op0=ALU.add)

        # points = sum_k traj_k (fp32); 1/4 folded into exp scale
        pts_f = pers.tile([128, CC, P], F32R, tag="ptsf")
        for cc in range(CC):
            tp = pstp.tile([128, 512], F32, tag="tp")
            for jb in range(4):
                nc.tensor.transpose(tp[:, jb * 128:(jb + 1) * 128],
                                    spts[:, jb, cc * 128:(cc + 1) * 128], idf_t)
            nc.scalar.activation(pts_f[:, cc, :], tp, AF.Copy)
        ptsr = pts_f

        # conv over t (3 taps) + bias3 + relu -> tc bf16
        tc_bf = pers.tile([128, CC, P], BF16, tag="tcbf")
        for ct in range(CC):
            c_ps = ps.tile([128, P], F32, tag="mm")
            cs = slice(ct * 128, (ct + 1) * 128)
            nc.tensor.matmul(c_ps, lhsT=wtT_t[:, 1, cs], rhs=fm,
                             start=True, stop=False)
            nc.tensor.matmul(c_ps[:, R:P], lhsT=wtT_t[:, 0, cs], rhs=fm[:, 0:P - R],
                             start=False, stop=False)
            nc.tensor.matmul(c_ps[:, 0:P - R], lhsT=wtT_t[:, 2, cs], rhs=fm[:, R:P],
                             start=False, stop=True)
            nc.scalar.activation(tc_bf[:, ct, :], c_ps, AF.Relu,
                                 bias=b3_t[:, ct:ct + 1])
        # tcT: (p, c) layout for prop matmuls
        tcT = pers.tile([128, CC, C], BF16, tag="tcT")
        for pb in range(4):
            tp2 = pstp.tile([128, 512], BF16, tag="tp")
            for cc in range(CC):
                nc.tensor.transpose(tp2[:, cc * 128:(cc + 1) * 128],
                                    tc_bf[:, cc, pb * 128:(pb + 1) * 128], idb_t)
            nc.vector.tensor_copy(tcT[:, pb, :], tp2)

        # ---- phase 4: z (s,p) -> softmax -> prop -> out, per s-tile ----
        # z per s-tile: lhsT = x slices, rhs = pts (f32r).  exp with
        # per-tile max bias + accumulated denominator; e transposed via PE
        # (bf16) and fed back as prop lhsT; normalize + residual fused in
        # one DVE pass at the output.
        for i in range(NCH):
            cw = min(P, S - i * P)
            nt = cw // 128
            for t in range(nt):
                st = 4 * i + t
                xres = sb3.tile([128, C], F32, tag="xres")
                nc.sync.dma_start(out=xres, in_=XSCr[:, st, 0:C])
                z_ps = ps.tile([128, P], F32, tag="mm")
                for cc in range(CC):
                    nc.tensor.matmul(z_ps,
                                     lhsT=xcr[:, cc, st * 128:(st + 1) * 128],
                                     rhs=ptsr[:, cc, :],
                                     start=(cc == 0), stop=(cc == CC - 1))
                nm = sb3.tile([128, 1], F32, tag="nm")
                nc.vector.tensor_reduce(nm, z_ps, axis=AX.X, op=ALU.max,
                                        negate=True)
                nm4 = sb3.tile([128, 1], F32, tag="nm4")
                nc.vector.tensor_scalar(nm4, nm, 0.25, None, op0=ALU.mult)
                e_sb = sb3.tile([128, P], BF16, tag="esb")
                dsum = sb3.tile([128, 1], F32, tag="dsum")
                nc.scalar.activation(e_sb, z_ps, AF.Exp, bias=nm4, scale=0.25,
                                     accum_out=dsum)
                eT_ps = pstp.tile([128, 512], BF16, tag="tp")
                for pb in range(4):
                    nc.tensor.transpose(eT_ps[:, pb * 128:(pb + 1) * 128],
                                        e_sb[:, pb * 128:(pb + 1) * 128], idb_t)
                eT = sb3.tile([128, 512], BF16, tag="eT")
                nc.scalar.activation(eT, eT_ps, AF.Copy)
                pr = pspr.tile([128, C], F32, tag="pr")
                for pb in range(4):
                    nc.tensor.matmul(pr, lhsT=eT[:, pb * 128:(pb + 1) * 128],
                                     rhs=tcT[:, pb, :],
                                     start=(pb == 0), stop=(pb == CC - 1))
                rd = sb3.tile([128, 1], F32, tag="rd")
                nc.vector.reciprocal(rd, dsum)
                osb = sb3.tile([128, C], F32, tag="osb")
                nc.vector.scalar_tensor_tensor(
                    osb, in0=pr, scalar=rd, in1=xres,
                    op0=ALU.mult, op1=ALU.add)
                nc.sync.dma_start(out=OUTr[:, st, :], in_=osb)
        ctx.close()
    nc.compile()
    return nc


def _host_prep(inputs):
    eps = 1e-5
    f32 = np.float32
    import ml_dtypes
    bf16 = ml_dtypes.bfloat16
    x = np.asarray(inputs["input"], f32)                       # (B,C,T,H,W)
    s1 = np.asarray(inputs["bn1_gamma"]) / np.sqrt(np.asarray(inputs["bn1_var"]) + eps)
    wrT = (np.asarray(inputs["w_reduce"], f32) * s1[:, None]).T.astype(f32)
    s2 = np.asarray(inputs["bn2_gamma"]) / np.sqrt(np.asarray(inputs["bn2_var"]) + eps)
    wp = np.asarray(inputs["w_proj"], f32) * s2[:, None]       # (Cq, C+2)
    b2 = (np.asarray(inputs["bn2_beta"])
          - np.asarray(inputs["bn2_mean"]) * s2).astype(f32)
    s3 = np.asarray(inputs["bn3_gamma"]) / np.sqrt(np.asarray(inputs["bn3_var"]) + eps)
    wt = np.asarray(inputs["w_t"], f32)[:, :, :, 0] * s3[:, None, None]  # (C,Cq,3)
    b3 = (np.asarray(inputs["bn3_beta"])
          - np.asarray(inputs["bn3_mean"]) * s3).astype(f32)
    common = {
        "wrT": np.ascontiguousarray(wrT),
        "wpT": np.ascontiguousarray(wp[:, :C].T.astype(bf16)),
        "wpc": np.ascontiguousarray(wp[:, C:].T.astype(bf16)),
        "wtT": np.ascontiguousarray(np.transpose(wt, (2, 1, 0)).astype(bf16)),
        "b2": b2.reshape(Cq, 1),
        "b3": b3.reshape(CC, 128),
        "identbf": np.eye(128, dtype=bf16),
        "identf": np.eye(128, dtype=f32),
        "iota128": np.arange(128, dtype=f32).reshape(128, 1),
    }
    x_cs = x.reshape(B, C, S)
    # augmented (S, CE) per-sample: x^T | row/H | col/W | zero pad
    hw_idx = np.arange(HW, dtype=f32)
    rowv = np.tile((hw_idx // W) / H, T)                       # (S,)
    colv = np.tile((hw_idx % W) / W, T)
    in_maps = []
    for b in range(B):
        m = dict(common)
        m["x_cs"] = np.ascontiguousarray(x_cs[b])
        m["x_sc"] = np.ascontiguousarray(x_cs[b].T)
        augb = np.zeros((S, CEB), bf16)
        augb[:, :C] = x_cs[b].T.astype(bf16)
        augb[:, C] = rowv.astype(bf16)
        augb[:, C + 1] = colv.astype(bf16)
        m["x_sb"] = augb
        in_maps.append(m)
    return in_maps


def kernel(**inputs) -> np.ndarray:
    if "nc" not in _CACHED:
        _CACHED["nc"] = build_nc()
    nc = _CACHED["nc"]
    in_maps = _host_prep(inputs)
    res = run_bass_kernel_spmd(nc, in_maps, list(range(B)))
    out = np.stack([res.results[b]["out_sc"].T for b in range(B)], axis=0)
    return out.reshape(B, C, T, H, W).astype(np.float32)


# revision 34
# speedup vs baseline: 1.0386x; 1.0251x over previous
"""Trainium2 Bass kernel for nn_CorrTrajBlock (sparse_attention).

Data-parallel over batch B=8 across 8 NeuronCores; one sample per core.

Per-core pipeline (C=512, T=8, H=W=28, HW=784, S=T*HW=6272, R=64, K=4,
Cq=128, P=T*R=512):
  1. template_p = w_reduce_eff @ x[:, 0]       (f32r matmul, 64x784)
     spt_inds = argmax over HW                 (DVE max/max_index)
  2. tres via on-chip one-hot selection (iota IS_EQUAL + f32r matmul
     against frame-0 x_sc chunks) - no DMA on the argmax->tres path
  3. affinity = tres^T @ x per t               (f32r matmul, 64x784 each)
     topk4 per (r, t) over HW                  (DVE max/max_index)
  4. topk idx wrapped layout built on-chip (PE fp32 transposes + i16
     copies), replicated 8x16 partitions; 4 bf16 dma_gathers of 512 rows
     (640 bf16 each, coords baked at cols 512:514)
  5. fuse = w_proj_eff @ [traj; coords] (bf16) -> max over k -> +bias2
     tc = relu(conv_t(fuse) + bias3) (bf16); tcT = (p, c) via PE transp
     points = sum_k traj (DVE, fp32) -> pts (c, p) f32r via PE transp
  6. per s-tile: z[s, p] = x-slice^T @ pts     (f32r, N=512)
     e = exp((z - max)/4) bf16 + accum denominator (ACT, bias/accum_out)
     eT via 4 PE transposes; prop[s, c] = eT^T @ tcT (bf16)
     out[s, c] = prop * (1/d) + x_sc           (one DVE stt pass)
     output written (S, C); host transposes back to (C, T, H, W).
"""
import sys

sys.path.insert(0, "/opt/trn_rl_repo")

import numpy as np
import concourse.bass as bass
import concourse.mybir as mybir
import concourse.tile as tile
from concourse import bacc
from concourse.bass_utils import run_bass_kernel_spmd

F32 = mybir.dt.float32
F32R = mybir.dt.float32r
BF16 = mybir.dt.bfloat16
I16 = mybir.dt.int16
I32 = mybir.dt.int32
U32 = mybir.dt.uint32
AF = mybir.ActivationFunctionType
ALU = mybir.AluOpType
AX = mybir.AxisListType

B, C, T, H, W = 8, 512, 8, 28, 28
HW = H * W            # 784
S = T * HW            # 6272
R = 64
K = 4
Cq = 128
P = T * R             # 512
CC = C // 128         # 4
CE = 576              # (unused fp32 row pad, kept for reference)
CEB = 640             # bf16 gather row: 512 x + 2 coords + pad (256B align)
NST = S // 128        # 49 s-tiles
NCH = 13              # s-chunks: 12 x 512 + 1 x 128

# affinity matmul dtype: f32r (fast) vs f32 (exact baseline fallback)
AFF_F32R = True
# pass 16-partition idx tiles to dma_gather directly (no 8x replication)
NO_REPL = False

_CACHED = {}


def build_nc():
    nc = bacc.Bacc("TRN2", debug=False)

    X_CS = nc.dram_tensor("x_cs", [C, S], F32, kind="ExternalInput").ap()
    X_SC = nc.dram_tensor("x_sc", [S, C], F32, kind="ExternalInput").ap()
    X_SB = nc.dram_tensor("x_sb", [S, CEB], BF16, kind="ExternalInput").ap()
    WRT = nc.dram_tensor("wrT", [C, R], F32, kind="ExternalInput").ap()
    WPT = nc.dram_tensor("wpT", [C, Cq], BF16, kind="ExternalInput").ap()
    WPC = nc.dram_tensor("wpc", [2, Cq], BF16, kind="ExternalInput").ap()
    WTT = nc.dram_tensor("wtT", [3, Cq, C], BF16, kind="ExternalInput").ap()
    B2 = nc.dram_tensor("b2", [Cq, 1], F32, kind="ExternalInput").ap()
    B3 = nc.dram_tensor("b3", [CC, 128], F32, kind="ExternalInput").ap()
    IDB = nc.dram_tensor("identbf", [128, 128], BF16, kind="ExternalInput").ap()
    IDF = nc.dram_tensor("identf", [128, 128], F32, kind="ExternalInput").ap()
    IOTA = nc.dram_tensor("iota128", [128, 1], F32, kind="ExternalInput").ap()
    OUT = nc.dram_tensor("out_sc", [S, C], F32, kind="ExternalOutput").ap()


    Xr = X_CS.rearrange("(cc p) s -> p cc s", p=128)
    XSCr = X_SC.rearrange("(n p) c -> p n c", p=128)
    OUTr = OUT.rearrange("(n p) c -> p n c", p=128)

    with tile.TileContext(nc) as tc:
        import contextlib
        ctx = contextlib.ExitStack()
        pers = ctx.enter_context(tc.tile_pool(name="pers", bufs=1))
        sb = ctx.enter_context(tc.tile_pool(name="sb", bufs=2))
        sb3 = ctx.enter_context(tc.tile_pool(name="sb3", bufs=3))
        sbg = ctx.enter_context(tc.tile_pool(name="sbg", bufs=3))
        ps = ctx.enter_context(tc.tile_pool(name="ps", bufs=3, space="PSUM"))
        pstp = ctx.enter_context(tc.tile_pool(name="pstp", bufs=2, space="PSUM"))
        pspr = ctx.enter_context(tc.tile_pool(name="pspr", bufs=3, space="PSUM"))

        # ---- persistent loads: frame 0 + weights first ----
        wrT_t = pers.tile([128, CC, R], F32R, tag="wrT")
        nc.sync.dma_start(out=wrT_t,
                          in_=WRT.rearrange("(cc p) r -> p cc r", p=128).bitcast(F32R))
        xc = pers.tile([128, CC, S], F32R, tag="xc")
        Xrr = Xr.bitcast(F32R)
        nc.sync.dma_start(out=xc[:, :, 0:392], in_=Xrr[:, :, 0:392])
        nc.sync.dma_start(out=xc[:, :, 392:HW], in_=Xrr[:, :, 392:HW])
        xcr = xc
        wpT_t = pers.tile([128, CC, Cq], BF16, tag="wpT")
        nc.sync.dma_start(out=wpT_t, in_=WPT.rearrange("(cc p) q -> p cc q", p=128))
        wpc_t = pers.tile([2, Cq], BF16, tag="wpc")
        nc.sync.dma_start(out=wpc_t, in_=WPC)
        wtT_t = pers.tile([128, 3, C], BF16, tag="wtT")
        nc.sync.dma_start(out=wtT_t, in_=WTT.rearrange("d p c -> p d c"))
        b2_t = pers.tile([128, 1], F32, tag="b2")
        nc.sync.dma_start(out=b2_t, in_=B2)
        b3_t = pers.tile([128, CC], F32, tag="b3")
        nc.sync.dma_start(out=b3_t, in_=B3.rearrange("cc p -> p cc"))
        idb_t = pers.tile([128, 128], BF16, tag="idb")
        nc.sync.dma_start(out=idb_t, in_=IDB)
        idf_t = pers.tile([128, 128], F32, tag="idf")
        nc.sync.dma_start(out=idf_t, in_=IDF)
        iota_t = pers.tile([128, 1], F32, tag="iota")
        nc.sync.dma_start(out=iota_t, in_=IOTA)
        xf0T = pers.tile([112, 7, C], F32R, tag="xf0T")
        nc.sync.dma_start(
            out=xf0T,
            in_=X_SC[0:HW, 0:C].rearrange("(j p) c -> p j c", p=112).bitcast(F32R))
        onesw = pers.tile([1, 128], F32, tag="onesw")
        nc.vector.memset(onesw, 1.0)

        # ---- phase 1: template + argmax ----
        tpl_sb = sb3.tile([64, HW], F32, tag="aff")
        for h in range(2):
            tp_ps = ps.tile([64, 392], F32, tag="mm")
            for cc in range(CC):
                nc.tensor.matmul(tp_ps,
                                 lhsT=wrT_t[:, cc, :],
                                 rhs=xcr[:, cc, h * 392:(h + 1) * 392],
                                 start=(cc == 0), stop=(cc == CC - 1))
            nc.scalar.activation(tpl_sb[:, h * 392:(h + 1) * 392], tp_ps, AF.Copy)
        for wi in range(16):
            wps = ps.tile([64, 392], F32, tag="mm")
            nc.tensor.matmul(wps, lhsT=wrT_t[:, wi % CC, :],
                             rhs=xcr[:, wi % CC, 0:392], start=True, stop=True)
        tmx = pers.tile([64, 8], F32, tag="tmx")
        tmi = pers.tile([64, 8], U32, tag="tmi")
        nc.vector.max(out=tmx, in_=tpl_sb)
        nc.vector.max_index(out=tmi, in_max=tmx, in_values=tpl_sb)
        spt_f = pers.tile([64, 1], F32, tag="sptf")
        nc.vector.tensor_copy(spt_f, tmi[:, 0:1])

        # tres via on-chip one-hot selection: no DMA in the argmax->tres path.
        # spt row -> broadcast to 112 partitions (ones-matmul), compare with
        # iota per 112-row chunk of frame-0 x_sc, one-hot matmul selects cols.
        tp_s = pstp.tile([1, 512], F32, tag="tp")
        nc.tensor.transpose(tp_s[:, 0:64], spt_f, idf_t[0:64, 0:64])
        t1s = pers.tile([1, 64], F32, tag="t1s")
        nc.vector.tensor_copy(t1s, tp_s[:, 0:64])
        rep_ps = pstp.tile([128, 512], F32, tag="tp")
        nc.tensor.matmul(rep_ps[:, 0:64], lhsT=onesw, rhs=t1s,
                         start=True, stop=True)
        rep = pers.tile([128, 64], F32, tag="rep")
        nc.scalar.activation(rep, rep_ps[:, 0:64], AF.Copy)
        tresT2_ps = pstp.tile([64, 512], F32, tag="tp")
        oh = []
        for j in range(7):
            oh_j = sb.tile([112, 64], F32, tag="oh")
            nc.vector.tensor_scalar(oh_j, rep[0:112, :], float(-112 * j),
                                    iota_t[0:112, :], op0=ALU.add,
                                    op1=ALU.is_equal)
            oh_r = sb.tile([112, 64], F32R, tag="ohr")
            nc.scalar.activation(oh_r, oh_j, AF.Copy)
            nc.tensor.matmul(tresT2_ps, lhsT=oh_r, rhs=xf0T[:, j, :],
                             start=(j == 0), stop=(j == 6))
        tresT2 = pers.tile([64, C], F32, tag="tresT2")
        nc.scalar.activation(tresT2, tresT2_ps, AF.Copy)
        tres = pers.tile([128, CC, R], F32R, tag="tres")
        for cc in range(CC):
            tp = pstp.tile([128, 512], F32, tag="tp")
            nc.tensor.transpose(tp[:, 0:64],
                                tresT2[:, cc * 128:(cc + 1) * 128],
                                idf_t[0:64, 0:64])
            nc.scalar.activation(tres[:, cc, :], tp[:, 0:64], AF.Copy)

        # rest of x: per-half-frame DMAs so affinity can stream per t
        for tb in range(1, T):
            for hh in range(2):
                i0 = tb * HW + hh * 392
                nc.sync.dma_start(out=xc[:, :, i0:i0 + 392],
                                  in_=Xrr[:, :, i0:i0 + 392])

        # ---- phase 2: affinity + topk (per t; 64-partition tiles) ----
        # staging layout: gstage2[r, (k t)] so that dram j = 512k + 64t + r
        gstage2 = pers.tile([64, 32], F32, tag="gstage2")
        gs2_v = gstage2.rearrange("r (k t) -> r t k", t=8)
        tres_mm = tres if AFF_F32R else tres.bitcast(F32)
        xc_mm = xcr if AFF_F32R else xc.bitcast(F32)
        for t in range(T):
            aff_sb = sb3.tile([64, HW], F32, tag="aff")
            for h in range(2):
                a_ps = ps.tile([64, 392], F32, tag="mm")
                for cc in range(CC):
                    nc.tensor.matmul(
                        a_ps,
                        lhsT=tres_mm[:, cc, :],
                        rhs=xc_mm[:, cc, t * HW + h * 392: t * HW + (h + 1) * 392],
                        start=(cc == 0), stop=(cc == CC - 1))
                nc.scalar.activation(aff_sb[:, h * 392:(h + 1) * 392], a_ps, AF.Copy)
            amx = sb3.tile([64, 8], F32, tag="amx")
            ami = sb3.tile([64, 8], U32, tag="ami")
            nc.vector.max(out=amx, in_=aff_sb)
            nc.vector.max_index(out=ami, in_max=amx, in_values=aff_sb)
            nc.vector.tensor_scalar(gs2_v[:, t, :], ami[:, 0:K],
                                    float(t * HW), None, op0=ALU.add)

        # build wrapped idx layout on-chip: gstage2[r=16rh+p16, q=8k+t]
        # -> w16[p16, 4q+rh] via PE int16 transposes, then replicate 8x.
        t1_ps = pstp.tile([32, 64], F32, tag="tp")
        nc.tensor.transpose(t1_ps, gstage2, idf_t[0:64, 0:64])
        t1 = pers.tile([32, 64], F32, tag="t1")
        nc.vector.tensor_copy(t1, t1_ps)
        w16 = pers.tile([16, 128], I16, tag="w16")
        w16v = w16.rearrange("p (q rh) -> p q rh", rh=4)
        for rh in range(4):
            wr_ps = pstp.tile([16, 32], F32, tag="tp")
            nc.tensor.transpose(wr_ps, t1[:, 16 * rh:16 * (rh + 1)],
                                idf_t[0:32, 0:32])
            nc.vector.tensor_copy(w16v[:, :, rh], wr_ps)
        if NO_REPL:
            gidx2 = w16
        else:
            gidx2 = pers.tile([128, 128], I16, tag="gidx2")
            for g in range(8):
                eng = nc.scalar if g % 2 == 0 else nc.sync
                eng.dma_start(out=gidx2[16 * g:16 * (g + 1), :], in_=w16)

        # ---- phase 3: traj gathers, fuse, points, conv ----
        fm_f32 = pers.tile([128, P], F32, tag="fmf")
        spts = pers.tile([128, 4, P], F32, tag="spts")
        gk = []
        for k in range(K):
            gk_t = sbg.tile([128, 4, CEB], BF16, tag="gk")
            gk.append(gk_t)
            if k == 0:
                for hg in range(2):
                    nc.gpsimd.dma_gather(
                        out_ap=gk_t[:, 2 * hg:2 * hg + 2, :], in_ap=X_SB,
                        idxs_ap=gidx2[:, 16 * hg:16 * hg + 16],
                        num_idxs=256, num_idxs_reg=256, elem_size=CEB)
            else:
                nc.gpsimd.dma_gather(out_ap=gk_t, in_ap=X_SB,
                                     idxs_ap=gidx2[:, k * 32:(k + 1) * 32],
                                     num_idxs=512, num_idxs_reg=512,
                                     elem_size=CEB)
            trajk = sb.tile([128, CC, P], BF16, tag="trajk")
            for cc in range(CC):
                tp = pstp.tile([128, 512], F32, tag="tp")
                tpb = tp.bitcast(BF16)
                for jb in range(4):
                    nc.tensor.transpose(tpb[:, jb * 128:(jb + 1) * 128],
                                        gk_t[:, jb, cc * 128:(cc + 1) * 128], idb_t)
                nc.scalar.activation(trajk[:, cc, :], tpb[:, 0:P], AF.Copy)
            # coords rows (gathered cols 512:514) -> (2, P)
            cd = pstp.tile([2, 512], F32, tag="tp")
            cdb = cd.bitcast(BF16)
            for jb in range(4):
                nc.tensor.transpose(cdb[:, jb * 128:(jb + 1) * 128],
                                    gk_t[:, jb, 512:514], idb_t)
            coordk = sb.tile([2, P], BF16, tag="coordk")
            nc.vector.tensor_copy(coordk, cdb[:, 0:P])
            f_ps = ps.tile([128, P], F32, tag="mm")
            for cc in range(CC):
                nc.tensor.matmul(f_ps, lhsT=wpT_t[:, cc, :], rhs=trajk[:, cc, :],
                                 start=(cc == 0), stop=False)
            nc.tensor.matmul(f_ps, lhsT=wpc_t, rhs=coordk,
                             start=False, stop=True)
            if k == 0:
                nc.scalar.activation(fm_f32, f_ps, AF.Copy)
            else:
                nc.vector.tensor_tensor(out=fm_f32, in0=fm_f32, in1=f_ps, op=ALU.max)
            if k == 1:
                nc.vector.tensor_tensor(out=spts, in0=gk[0][:, :, 0:P],
                                        in1=gk[1][:, :, 0:P], op=ALU.add)
            elif k > 1:
                nc.vector.tensor_tensor(out=spts, in0=spts,
                                        in1=gk_t[:, :, 0:P], op=ALU.add)
        fm = pers.tile([128, P], BF16, tag="fm")
        nc.vector.tensor_scalar(fm, fm_f32, b2_t, None, op0=ALU.add)

        # points = sum_k traj_k (fp32); 1/4 folded into exp scale
        pts_f = pers.tile([128, CC, P], F32R, tag="ptsf")
        for cc in range(CC):
            tp = pstp.tile([128, 512], F32, tag="tp")
            for jb in range(4):
                nc.tensor.transpose(tp[:, jb * 128:(jb + 1) * 128],
                                    spts[:, jb, cc * 128:(cc + 1) * 128], idf_t)
            nc.scalar.activation(pts_f[:, cc, :], tp, AF.Copy)
        ptsr = pts_f

        # conv over t (3 taps) + bias3 + relu -> tc bf16
        tc_bf = pers.tile([128, CC, P], BF16, tag="tcbf")
        for ct in range(CC):
            c_ps = ps.tile([128, P], F32, tag="mm")
            cs = slice(ct * 128, (ct + 1) * 128)
            nc.tensor.matmul(c_ps, lhsT=wtT_t[:, 1, cs], rhs=fm,
                             start=True, stop=False)
            nc.tensor.matmul(c_ps[:, R:P], lhsT=wtT_t[:, 0, cs], rhs=fm[:, 0:P - R],
                             start=False, stop=False)
            nc.tensor.matmul(c_ps[:, 0:P - R], lhsT=wtT_t[:, 2, cs], rhs=fm[:, R:P],
                             start=False, stop=True)
            nc.scalar.activation(tc_bf[:, ct, :], c_ps, AF.Relu,
                                 bias=b3_t[:, ct:ct + 1])
        # tcT: (p, c) layout for prop matmuls
        tcT = pers.tile([128, CC, C], BF16, tag="tcT")
        for pb in range(4):
            tp2 = pstp.tile([128, 512], BF16, tag="tp")
            for cc in range(CC):
                nc.tensor.transpose(tp2[:, cc * 128:(cc + 1) * 128],
                                    tc_bf[:, cc, pb * 128:(pb + 1) * 128], idb_t)
            nc.vector.tensor_copy(tcT[:, pb, :], tp2)

        # ---- phase 4: z (s,p) -> softmax -> prop -> out, per s-tile ----
        # z per s-tile: lhsT = x slices, rhs = pts (f32r).  exp with
        # per-tile max bias + accumulated denominator; e transposed via PE
        # (bf16) and fed back as prop lhsT; normalize + residual fused in
        # one DVE pass at the output.
        for i in range(NCH):
            cw = min(P, S - i * P)
            nt = cw // 128
            for t in range(nt):
                st = 4 * i + t
                xres = sb3.tile([128, C], F32, tag="xres")
                nc.sync.dma_start(out=xres, in_=XSCr[:, st, 0:C])
                z_ps = ps.tile([128, P], F32, tag="mm")
                for cc in range(CC):
                    nc.tensor.matmul(z_ps,
                                     lhsT=xcr[:, cc, st * 128:(st + 1) * 128],
                                     rhs=ptsr[:, cc, :],
                                     start=(cc == 0), stop=(cc == CC - 1))
                nm = sb3.tile([128, 1], F32, tag="nm")
                nc.vector.tensor_reduce(nm, z_ps, axis=AX.X, op=ALU.max,
                                        negate=True)
                nm4 = sb3.tile([128, 1], F32, tag="nm4")
                nc.vector.tensor_scalar(nm4, nm, 0.25, None, op0=ALU.mult)
                e_sb = sb3.tile([128, P], BF16, tag="esb")
                dsum = sb3.tile([128, 1], F32, tag="dsum")
                nc.scalar.activation(e_sb, z_ps, AF.Exp, bias=nm4, scale=0.25,
                                     accum_out=dsum)
                eT_ps = pstp.tile([128, 512], BF16, tag="tp")
                for pb in range(4):
                    nc.tensor.transpose(eT_ps[:, pb * 128:(pb + 1) * 128],
                                        e_sb[:, pb * 128:(pb + 1) * 128], idb_t)
                eT = sb3.tile([128, 512], BF16, tag="eT")
                nc.scalar.activation(eT, eT_ps, AF.Copy)
                pr = pspr.tile([128, C], F32, tag="pr")
                for pb in range(4):
                    nc.tensor.matmul(pr, lhsT=eT[:, pb * 128:(pb + 1) * 128],
                                     rhs=tcT[:, pb, :],
                                     start=(pb == 0), stop=(pb == CC - 1))
                rd = sb3.tile([128, 1], F32, tag="rd")
                nc.vector.reciprocal(rd, dsum)
                osb = sb3.tile([128, C], F32, tag="osb")
                nc.vector.scalar_tensor_tensor(
                    osb, in0=pr, scalar=rd, in1=xres,
                    op0=ALU.mult, op1=ALU.add)
                nc.sync.dma_start(out=OUTr[:, st, :], in_=osb)
        ctx.close()
    nc.compile()
    return nc


def _host_prep(inputs):
    eps = 1e-5
    f32 = np.float32
    import ml_dtypes
    bf16 = ml_dtypes.bfloat16
    x = np.asarray(inputs["input"], f32)                       # (B,C,T,H,W)
    s1 = np.asarray(inputs["bn1_gamma"]) / np.sqrt(np.asarray(inputs["bn1_var"]) + eps)
    wrT = (np.asarray(inputs["w_reduce"], f32) * s1[:, None]).T.astype(f32)
    s2 = np.asarray(inputs["bn2_gamma"]) / np.sqrt(np.asarray(inputs["bn2_var"]) + eps)
    wp = np.asarray(inputs["w_proj"], f32) * s2[:, None]       # (Cq, C+2)
    b2 = (np.asarray(inputs["bn2_beta"])
          - np.asarray(inputs["bn2_mean"]) * s2).astype(f32)
    s3 = np.asarray(inputs["bn3_gamma"]) / np.sqrt(np.asarray(inputs["bn3_var"]) + eps)
    wt = np.asarray(inputs["w_t"], f32)[:, :, :, 0] * s3[:, None, None]  # (C,Cq,3)
    b3 = (np.asarray(inputs["bn3_beta"])
          - np.asarray(inputs["bn3_mean"]) * s3).astype(f32)
    common = {
        "wrT": np.ascontiguousarray(wrT),
        "wpT": np.ascontiguousarray(wp[:, :C].T.astype(bf16)),
        "wpc": np.ascontiguousarray(wp[:, C:].T.astype(bf16)),
        "wtT": np.ascontiguousarray(np.transpose(wt, (2, 1, 0)).astype(bf16)),
        "b2": b2.reshape(Cq, 1),
        "b3": b3.reshape(CC, 128),
        "identbf": np.eye(128, dtype=bf16),
        "identf": np.eye(128, dtype=f32),
        "iota128": np.arange(128, dtype=f32).reshape(128, 1),
    }
    x_cs = x.reshape(B, C, S)
    # augmented (S, CE) per-sample: x^T | row/H | col/W | zero pad
    hw_idx = np.arange(HW, dtype=f32)
    rowv = np.tile((hw_idx // W) / H, T)                       # (S,)
    colv = np.tile((hw_idx % W) / W, T)
    in_maps = []
    for b in range(B):
        m = dict(common)
        m["x_cs"] = np.ascontiguousarray(x_cs[b])
        m["x_sc"] = np.ascontiguousarray(x_cs[b].T)
        augb = np.zeros((S, CEB), bf16)
        augb[:, :C] = x_cs[b].T.astype(bf16)
        augb[:, C] = rowv.astype(bf16)
        augb[:, C + 1] = colv.astype(bf16)
        m["x_sb"] = augb
        in_maps.append(m)
    return in_maps


def kernel(**inputs) -> np.ndarray:
    if "nc" not in _CACHED:
        _CACHED["nc"] = build_nc()
    nc = _CACHED["nc"]
    in_maps = _host_prep(inputs)
    res = run_bass_kernel_spmd(nc, in_maps, list(range(B)))
    out = np.stack([res.results[b]["out_sc"].T for b in range(B)], axis=0)
    return out.reshape(B, C, T, H, W).astype(np.float32)


# revision 35
# speedup vs baseline: 1.0689x; 1.0292x over previous
"""Trainium2 Bass kernel for nn_CorrTrajBlock (sparse_attention).

Data-parallel over batch B=8 across 8 NeuronCores; one sample per core.

Per-core pipeline (C=512, T=8, H=W=28, HW=784, S=T*HW=6272, R=64, K=4,
Cq=128, P=T*R=512):
  1. template_p = w_reduce_eff @ x[:, 0]       (f32r matmul, 64x784)
     spt_inds = argmax over HW                 (DVE max/max_index)
  2. tres via on-chip one-hot selection (iota IS_EQUAL + f32r matmul
     against frame-0 x_sc chunks) - no DMA on the argmax->tres path
  3. affinity = tres^T @ x per t               (f32r matmul, 64x784 each)
     topk4 per (r, t) over HW                  (DVE max/max_index)
  4. topk idx wrapped layout built on-chip (PE fp32 transposes + i16
     copies), replicated 8x16 partitions; 4 bf16 dma_gathers of 512 rows
     (640 bf16 each, coords baked at cols 512:514)
  5. fuse = w_proj_eff @ [traj; coords] (bf16) -> max over k -> +bias2
     tc = relu(conv_t(fuse) + bias3) (bf16); tcT = (p, c) via PE transp
     points = sum_k traj (DVE, fp32) -> pts (c, p) f32r via PE transp
  6. per s-tile: z[s, p] = x-slice^T @ pts     (f32r, N=512)
     e = exp((z - max)/4) bf16 + accum denominator (ACT, bias/accum_out)
     eT via 4 PE transposes; prop[s, c] = eT^T @ tcT (bf16)
     out[s, c] = prop * (1/d) + x_sc           (one DVE stt pass)
     output written (S, C); host transposes back to (C, T, H, W).
"""
import sys

sys.path.insert(0, "/opt/trn_rl_repo")

import numpy as np
import concourse.bass as bass
import concourse.mybir as mybir
import concourse.tile as tile
from concourse import bacc
from concourse.bass_utils import run_bass_kernel_spmd

F32 = mybir.dt.float32
F32R = mybir.dt.float32r
BF16 = mybir.dt.bfloat16
I16 = mybir.dt.int16
I32 = mybir.dt.int32
U32 = mybir.dt.uint32
AF = mybir.ActivationFunctionType
ALU = mybir.AluOpType
AX = mybir.AxisListType

B, C, T, H, W = 8, 512, 8, 28, 28
HW = H * W            # 784
S = T * HW            # 6272
R = 64
K = 4
Cq = 128
P = T * R             # 512
CC = C // 128         # 4
CE = 576              # (unused fp32 row pad, kept for reference)
CEB = 640             # bf16 gather row: 512 x + 2 coords + pad (256B align)
NST = S // 128        # 49 s-tiles
NCH = 13              # s-chunks: 12 x 512 + 1 x 128

# affinity matmul dtype: f32r (fast) vs f32 (exact baseline fallback)
AFF_F32R = True
# pass 16-partition idx tiles to dma_gather directly (no 8x replication)
NO_REPL = False

_CACHED = {}


def build_nc():
    nc = bacc.Bacc("TRN2", debug=False)

    X_CS = nc.dram_tensor("x_cs", [C, S], F32, kind="ExternalInput").ap()
    X_SC = nc.dram_tensor("x_sc", [S, C], F32, kind="ExternalInput").ap()
    X_SB = nc.dram_tensor("x_sb", [S, CEB], BF16, kind="ExternalInput").ap()
    WRT = nc.dram_tensor("wrT", [C, R], F32, kind="ExternalInput").ap()
    WPT = nc.dram_tensor("wpT", [C, Cq], BF16, kind="ExternalInput").ap()
    WPC = nc.dram_tensor("wpc", [2, Cq], BF16, kind="ExternalInput").ap()
    WTT = nc.dram_tensor("wtT", [3, Cq, C], BF16, kind="ExternalInput").ap()
    B2 = nc.dram_tensor("b2", [Cq, 1], F32, kind="ExternalInput").ap()
    B3 = nc.dram_tensor("b3", [CC, 128], F32, kind="ExternalInput").ap()
    IDB = nc.dram_tensor("identbf", [128, 128], BF16, kind="ExternalInput").ap()
    IDF = nc.dram_tensor("identf", [128, 128], F32, kind="ExternalInput").ap()
    IOTA = nc.dram_tensor("iota128", [128, 1], F32, kind="ExternalInput").ap()
    OUT = nc.dram_tensor("out_sc", [S, C], F32, kind="ExternalOutput").ap()


    Xr = X_CS.rearrange("(cc p) s -> p cc s", p=128)
    XSCr = X_SC.rearrange("(n p) c -> p n c", p=128)
    OUTr = OUT.rearrange("(n p) c -> p n c", p=128)

    with tile.TileContext(nc) as tc:
        import contextlib
        ctx = contextlib.ExitStack()
        pers = ctx.enter_context(tc.tile_pool(name="pers", bufs=1))
        sb = ctx.enter_context(tc.tile_pool(name="sb", bufs=2))
        sb3 = ctx.enter_context(tc.tile_pool(name="sb3", bufs=3))
        sbg = ctx.enter_context(tc.tile_pool(name="sbg", bufs=3))
        ps = ctx.enter_context(tc.tile_pool(name="ps", bufs=3, space="PSUM"))
        pstp = ctx.enter_context(tc.tile_pool(name="pstp", bufs=2, space="PSUM"))
        pspr = ctx.enter_context(tc.tile_pool(name="pspr", bufs=3, space="PSUM"))

        # ---- persistent loads: frame 0 + weights first ----
        wrT_t = pers.tile([128, CC, R], F32R, tag="wrT")
        nc.sync.dma_start(out=wrT_t,
                          in_=WRT.rearrange("(cc p) r -> p cc r", p=128).bitcast(F32R))
        xc = pers.tile([128, CC, S], F32R, tag="xc")
        Xrr = Xr.bitcast(F32R)
        nc.sync.dma_start(out=xc[:, :, 0:392], in_=Xrr[:, :, 0:392])
        nc.sync.dma_start(out=xc[:, :, 392:HW], in_=Xrr[:, :, 392:HW])
        xcr = xc
        wpT_t = pers.tile([128, CC, Cq], BF16, tag="wpT")
        nc.sync.dma_start(out=wpT_t, in_=WPT.rearrange("(cc p) q -> p cc q", p=128))
        wpc_t = pers.tile([2, Cq], BF16, tag="wpc")
        nc.sync.dma_start(out=wpc_t, in_=WPC)
        wtT_t = pers.tile([128, 3, C], BF16, tag="wtT")
        nc.sync.dma_start(out=wtT_t, in_=WTT.rearrange("d p c -> p d c"))
        b2_t = pers.tile([128, 1], F32, tag="b2")
        nc.sync.dma_start(out=b2_t, in_=B2)
        b3_t = pers.tile([128, CC], F32, tag="b3")
        nc.sync.dma_start(out=b3_t, in_=B3.rearrange("cc p -> p cc"))
        idb_t = pers.tile([128, 128], BF16, tag="idb")
        nc.sync.dma_start(out=idb_t, in_=IDB)
        idf_t = pers.tile([128, 128], F32, tag="idf")
        nc.sync.dma_start(out=idf_t, in_=IDF)
        iota_t = pers.tile([128, 1], F32, tag="iota")
        nc.sync.dma_start(out=iota_t, in_=IOTA)
        xf0T = pers.tile([112, 7, C], F32R, tag="xf0T")
        nc.sync.dma_start(
            out=xf0T,
            in_=X_SC[0:HW, 0:C].rearrange("(j p) c -> p j c", p=112).bitcast(F32R))
        onesw = pers.tile([1, 128], F32, tag="onesw")
        nc.vector.memset(onesw, 1.0)

        # ---- phase 1: template + argmax ----
        tpl_sb = sb3.tile([64, HW], F32, tag="aff")
        for h in range(2):
            tp_ps = ps.tile([64, 392], F32, tag="mm")
            for cc in range(CC):
                nc.tensor.matmul(tp_ps,
                                 lhsT=wrT_t[:, cc, :],
                                 rhs=xcr[:, cc, h * 392:(h + 1) * 392],
                                 start=(cc == 0), stop=(cc == CC - 1))
            nc.scalar.activation(tpl_sb[:, h * 392:(h + 1) * 392], tp_ps, AF.Copy)
        for wi in range(16):
            wps = ps.tile([64, 392], F32, tag="mm")
            nc.tensor.matmul(wps, lhsT=wrT_t[:, wi % CC, :],
                             rhs=xcr[:, wi % CC, 0:392], start=True, stop=True)
        tmx = pers.tile([64, 8], F32, tag="tmx")
        tmi = pers.tile([64, 8], U32, tag="tmi")
        nc.vector.max(out=tmx, in_=tpl_sb)
        nc.vector.max_index(out=tmi, in_max=tmx, in_values=tpl_sb)
        spt_f = pers.tile([64, 1], F32, tag="sptf")
        nc.vector.tensor_copy(spt_f, tmi[:, 0:1])

        # tres via on-chip one-hot selection: no DMA in the argmax->tres path.
        # spt row -> broadcast to 112 partitions (ones-matmul), compare with
        # iota per 112-row chunk of frame-0 x_sc, one-hot matmul selects cols.
        tp_s = pstp.tile([1, 512], F32, tag="tp")
        nc.tensor.transpose(tp_s[:, 0:64], spt_f, idf_t[0:64, 0:64])
        t1s = pers.tile([1, 64], F32, tag="t1s")
        nc.vector.tensor_copy(t1s, tp_s[:, 0:64])
        rep_ps = pstp.tile([128, 512], F32, tag="tp")
        nc.tensor.matmul(rep_ps[:, 0:64], lhsT=onesw, rhs=t1s,
                         start=True, stop=True)
        rep = pers.tile([128, 64], F32, tag="rep")
        nc.scalar.activation(rep, rep_ps[:, 0:64], AF.Copy)
        tresT2_ps = pstp.tile([64, 512], F32, tag="tp")
        oh = []
        for j in range(7):
            oh_j = sb.tile([112, 64], F32, tag="oh")
            nc.vector.tensor_scalar(oh_j, rep[0:112, :], float(-112 * j),
                                    iota_t[0:112, :], op0=ALU.add,
                                    op1=ALU.is_equal)
            oh_r = sb.tile([112, 64], F32R, tag="ohr")
            nc.scalar.activation(oh_r, oh_j, AF.Copy)
            nc.tensor.matmul(tresT2_ps, lhsT=oh_r, rhs=xf0T[:, j, :],
                             start=(j == 0), stop=(j == 6))
        tresT2 = pers.tile([64, C], F32, tag="tresT2")
        nc.scalar.activation(tresT2, tresT2_ps, AF.Copy)
        tres = pers.tile([128, CC, R], F32R, tag="tres")
        for cc in range(CC):
            tp = pstp.tile([128, 512], F32, tag="tp")
            nc.tensor.transpose(tp[:, 0:64],
                                tresT2[:, cc * 128:(cc + 1) * 128],
                                idf_t[0:64, 0:64])
            nc.scalar.activation(tres[:, cc, :], tp[:, 0:64], AF.Copy)

        # rest of x: per-half-frame DMAs so affinity can stream per t
        for tb in range(1, T):
            for hh in range(2):
                i0 = tb * HW + hh * 392
                nc.sync.dma_start(out=xc[:, :, i0:i0 + 392],
                                  in_=Xrr[:, :, i0:i0 + 392])

        # ---- phase 2: affinity + topk (per t; 64-partition tiles) ----
        # staging layout: gstage2[r, (k t)] so that dram j = 512k + 64t + r
        gstage2 = pers.tile([64, 32], F32, tag="gstage2")
        gs2_v = gstage2.rearrange("r (k t) -> r t k", t=8)
        tres_mm = tres if AFF_F32R else tres.bitcast(F32)
        xc_mm = xcr if AFF_F32R else xc.bitcast(F32)
        for t in range(T):
            aff_sb = sb3.tile([64, HW], F32, tag="aff")
            for h in range(2):
                a_ps = ps.tile([64, 392], F32, tag="mm")
                for cc in range(CC):
                    nc.tensor.matmul(
                        a_ps,
                        lhsT=tres_mm[:, cc, :],
                        rhs=xc_mm[:, cc, t * HW + h * 392: t * HW + (h + 1) * 392],
                        start=(cc == 0), stop=(cc == CC - 1))
                nc.scalar.activation(aff_sb[:, h * 392:(h + 1) * 392], a_ps, AF.Copy)
            amx = sb3.tile([64, 8], F32, tag="amx")
            ami = sb3.tile([64, 8], U32, tag="ami")
            nc.vector.max(out=amx, in_=aff_sb)
            nc.vector.max_index(out=ami, in_max=amx, in_values=aff_sb)
            nc.vector.tensor_scalar(gs2_v[:, t, :], ami[:, 0:K],
                                    float(t * HW), None, op0=ALU.add)

        # build wrapped idx layout on-chip: gstage2[r=16rh+p16, q=8k+t]
        # -> w16[p16, 4q+rh] via PE int16 transposes, then replicate 8x.
        t1_ps = pstp.tile([32, 64], F32, tag="tp")
        nc.tensor.transpose(t1_ps, gstage2, idf_t[0:64, 0:64])
        t1 = pers.tile([32, 64], F32, tag="t1")
        nc.vector.tensor_copy(t1, t1_ps)
        w16 = pers.tile([16, 128], I16, tag="w16")
        w16v = w16.rearrange("p (q rh) -> p q rh", rh=4)
        for rh in range(4):
            wr_ps = pstp.tile([16, 32], F32, tag="tp")
            nc.tensor.transpose(wr_ps, t1[:, 16 * rh:16 * (rh + 1)],
                                idf_t[0:32, 0:32])
            nc.vector.tensor_copy(w16v[:, :, rh], wr_ps)
        if NO_REPL:
            gidx2 = w16
        else:
            gidx2 = pers.tile([128, 128], I16, tag="gidx2")
            for g in range(8):
                eng = nc.scalar if g % 2 == 0 else nc.sync
                eng.dma_start(out=gidx2[16 * g:16 * (g + 1), :], in_=w16)

        # ---- phase 3: traj gathers, fuse, points, conv ----
        fm_f32 = pers.tile([128, P], F32, tag="fmf")
        spts = pers.tile([128, 4, P], F32, tag="spts")
        gk = []
        for k in range(K):
            gk_t = sbg.tile([128, 4, CEB], BF16, tag="gk")
            gk.append(gk_t)
            if k == 0:
                for hg in range(2):
                    nc.gpsimd.dma_gather(
                        out_ap=gk_t[:, 2 * hg:2 * hg + 2, :], in_ap=X_SB,
                        idxs_ap=gidx2[:, 16 * hg:16 * hg + 16],
                        num_idxs=256, num_idxs_reg=256, elem_size=CEB)
            else:
                nc.gpsimd.dma_gather(out_ap=gk_t, in_ap=X_SB,
                                     idxs_ap=gidx2[:, k * 32:(k + 1) * 32],
                                     num_idxs=512, num_idxs_reg=512,
                                     elem_size=CEB)
            trajk = sb.tile([128, CC, P], BF16, tag="trajk")
            for cc in range(CC):
                tp = pstp.tile([128, 512], F32, tag="tp")
                tpb = tp.bitcast(BF16)
                for jb in range(4):
                    nc.tensor.transpose(tpb[:, jb * 128:(jb + 1) * 128],
                                        gk_t[:, jb, cc * 128:(cc + 1) * 128], idb_t)
                nc.scalar.activation(trajk[:, cc, :], tpb[:, 0:P], AF.Copy)
            # coords rows (gathered cols 512:514) -> (2, P)
            cd = pstp.tile([2, 512], F32, tag="tp")
            cdb = cd.bitcast(BF16)
            for jb in range(4):
                nc.tensor.transpose(cdb[:, jb * 128:(jb + 1) * 128],
                                    gk_t[:, jb, 512:514], idb_t)
            coordk = sb.tile([2, P], BF16, tag="coordk")
            nc.vector.tensor_copy(coordk, cdb[:, 0:P])
            f_ps = ps.tile([128, P], F32, tag="mm")
            for cc in range(CC):
                nc.tensor.matmul(f_ps, lhsT=wpT_t[:, cc, :], rhs=trajk[:, cc, :],
                                 start=(cc == 0), stop=False)
            nc.tensor.matmul(f_ps, lhsT=wpc_t, rhs=coordk,
                             start=False, stop=True)
            if k == 0:
                nc.scalar.activation(fm_f32, f_ps, AF.Copy)
            else:
                nc.vector.tensor_tensor(out=fm_f32, in0=fm_f32, in1=f_ps, op=ALU.max)
            if k == 1:
                for jb in range(4):
                    nc.vector.tensor_tensor(out=spts[:, jb:jb + 1, :],
                                            in0=gk[0][:, jb:jb + 1, 0:P],
                                            in1=gk[1][:, jb:jb + 1, 0:P],
                                            op=ALU.add)
            elif k > 1:
                for jb in range(4):
                    nc.vector.tensor_tensor(out=spts[:, jb:jb + 1, :],
                                            in0=spts[:, jb:jb + 1, :],
                                            in1=gk_t[:, jb:jb + 1, 0:P],
                                            op=ALU.add)
        fm = pers.tile([128, P], BF16, tag="fm")
        nc.vector.tensor_scalar(fm, fm_f32, b2_t, None, op0=ALU.add)

        # points = sum_k traj_k (fp32); 1/4 folded into exp scale
        pts_f = pers.tile([128, CC, P], F32R, tag="ptsf")
        for cc in range(CC):
            tp = pstp.tile([128, 512], F32, tag="tp")
            for jb in range(4):
                nc.tensor.transpose(tp[:, jb * 128:(jb + 1) * 128],
                                    spts[:, jb, cc * 128:(cc + 1) * 128], idf_t)
            nc.scalar.activation(pts_f[:, cc, :], tp, AF.Copy)
        ptsr = pts_f

        # conv over t (3 taps) + bias3 + relu -> tc bf16
        tc_bf = pers.tile([128, CC, P], BF16, tag="tcbf")
        for ct in range(CC):
            c_ps = ps.tile([128, P], F32, tag="mm")
            cs = slice(ct * 128, (ct + 1) * 128)
            nc.tensor.matmul(c_ps, lhsT=wtT_t[:, 1, cs], rhs=fm,
                             start=True, stop=False)
            nc.tensor.matmul(c_ps[:, R:P], lhsT=wtT_t[:, 0, cs], rhs=fm[:, 0:P - R],
                             start=False, stop=False)
            nc.tensor.matmul(c_ps[:, 0:P - R], lhsT=wtT_t[:, 2, cs], rhs=fm[:, R:P],
                             start=False, stop=True)
            nc.scalar.activation(tc_bf[:, ct, :], c_ps, AF.Relu,
                                 bias=b3_t[:, ct:ct + 1])
        # tcT: (p, c) layout for prop matmuls
        tcT = pers.tile([128, CC, C], BF16, tag="tcT")
        for pb in range(4):
            tp2 = pstp.tile([128, 512], BF16, tag="tp")
            for cc in range(CC):
                nc.tensor.transpose(tp2[:, cc * 128:(cc + 1) * 128],
                                    tc_bf[:, cc, pb * 128:(pb + 1) * 128], idb_t)
            nc.vector.tensor_copy(tcT[:, pb, :], tp2)

        # ---- phase 4: z (s,p) -> softmax -> prop -> out, per s-tile ----
        # z per s-tile: lhsT = x slices, rhs = pts (f32r).  exp with
        # per-tile max bias + accumulated denominator; e transposed via PE
        # (bf16) and fed back as prop lhsT; normalize + residual fused in
        # one DVE pass at the output.
        for i in range(NCH):
            cw = min(P, S - i * P)
            nt = cw // 128
            for t in range(nt):
                st = 4 * i + t
                xres = sb3.tile([128, C], F32, tag="xres")
                nc.sync.dma_start(out=xres, in_=XSCr[:, st, 0:C])
                z_ps = ps.tile([128, P], F32, tag="mm")
                for cc in range(CC):
                    nc.tensor.matmul(z_ps,
                                     lhsT=xcr[:, cc, st * 128:(st + 1) * 128],
                                     rhs=ptsr[:, cc, :],
                                     start=(cc == 0), stop=(cc == CC - 1))
                nm = sb3.tile([128, 1], F32, tag="nm")
                nc.vector.tensor_reduce(nm, z_ps, axis=AX.X, op=ALU.max,
                                        negate=True)
                nm4 = sb3.tile([128, 1], F32, tag="nm4")
                nc.vector.tensor_scalar(nm4, nm, 0.25, None, op0=ALU.mult)
                e_sb = sb3.tile([128, P], BF16, tag="esb")
                dsum = sb3.tile([128, 1], F32, tag="dsum")
                nc.scalar.activation(e_sb, z_ps, AF.Exp, bias=nm4, scale=0.25,
                                     accum_out=dsum)
                eT_ps = pstp.tile([128, 512], BF16, tag="tp")
                for pb in range(4):
                    nc.tensor.transpose(eT_ps[:, pb * 128:(pb + 1) * 128],
                                        e_sb[:, pb * 128:(pb + 1) * 128], idb_t)
                eT = sb3.tile([128, 512], BF16, tag="eT")
                nc.scalar.activation(eT, eT_ps, AF.Copy)
                pr = pspr.tile([128, C], F32, tag="pr")
                for pb in range(4):
                    nc.tensor.matmul(pr, lhsT=eT[:, pb * 128:(pb + 1) * 128],
                                     rhs=tcT[:, pb, :],
                                     start=(pb == 0), stop=(pb == CC - 1))
                rd = sb3.tile([128, 1], F32, tag="rd")
                nc.vector.reciprocal(rd, dsum)
                osb = sb3.tile([128, C], F32, tag="osb")
                nc.vector.scalar_tensor_tensor(
                    osb, in0=pr, scalar=rd, in1=xres,
                    op0=ALU.mult, op1=ALU.add)
                nc.sync.dma_start(out=OUTr[:, st, :], in_=osb)
        ctx.close()
    nc.compile()
    return nc


def _host_prep(inputs):
    eps = 1e-5
    f32 = np.float32
    import ml_dtypes
    bf16 = ml_dtypes.bfloat16
    x = np.asarray(inputs["input"], f32)                       # (B,C,T,H,W)
    s1 = np.asarray(inputs["bn1_gamma"]) / np.sqrt(np.asarray(inputs["bn1_var"]) + eps)
    wrT = (np.asarray(inputs["w_reduce"], f32) * s1[:, None]).T.astype(f32)
    s2 = np.asarray(inputs["bn2_gamma"]) / np.sqrt(np.asarray(inputs["bn2_var"]) + eps)
    wp = np.asarray(inputs["w_proj"], f32) * s2[:, None]       # (Cq, C+2)
    b2 = (np.asarray(inputs["bn2_beta"])
          - np.asarray(inputs["bn2_mean"]) * s2).astype(f32)
    s3 = np.asarray(inputs["bn3_gamma"]) / np.sqrt(np.asarray(inputs["bn3_var"]) + eps)
    wt = np.asarray(inputs["w_t"], f32)[:, :, :, 0] * s3[:, None, None]  # (C,Cq,3)
    b3 = (np.asarray(inputs["bn3_beta"])
          - np.asarray(inputs["bn3_mean"]) * s3).astype(f32)
    common = {
        "wrT": np.ascontiguousarray(wrT),
        "wpT": np.ascontiguousarray(wp[:, :C].T.astype(bf16)),
        "wpc": np.ascontiguousarray(wp[:, C:].T.astype(bf16)),
        "wtT": np.ascontiguousarray(np.transpose(wt, (2, 1, 0)).astype(bf16)),
        "b2": b2.reshape(Cq, 1),
        "b3": b3.reshape(CC, 128),
        "identbf": np.eye(128, dtype=bf16),
        "identf": np.eye(128, dtype=f32),
        "iota128": np.arange(128, dtype=f32).reshape(128, 1),
    }
    x_cs = x.reshape(B, C, S)
    # augmented (S, CE) per-sample: x^T | row/H | col/W | zero pad
    hw_idx = np.arange(HW, dtype=f32)
    rowv = np.tile((hw_idx // W) / H, T)                       # (S,)
    colv = np.tile((hw_idx % W) / W, T)
    in_maps = []
    for b in range(B):
        m = dict(common)
        m["x_cs"] = np.ascontiguousarray(x_cs[b])
        m["x_sc"] = np.ascontiguousarray(x_cs[b].T)
        augb = np.zeros((S, CEB), bf16)
        augb[:, :C] = x_cs[b].T.astype(bf16)
        augb[:, C] = rowv.astype(bf16)
        augb[:, C + 1] = colv.astype(bf16)
        m["x_sb"] = augb
        in_maps.append(m)
    return in_maps


def kernel(**inputs) -> np.ndarray:
    if "nc" not in _CACHED:
        _CACHED["nc"] = build_nc()
    nc = _CACHED["nc"]
    in_maps = _host_prep(inputs)
    res = run_bass_kernel_spmd(nc, in_maps, list(range(B)))
    out = np.stack([res.results[b]["out_sc"].T for b in range(B)], axis=0)
    return out.reshape(B, C, T, H, W).astype(np.float32)
